# revision 17
# baseline (speedup 1.0000x reference)
"""
AdaptiveMessagePassingLayer Trainium2 kernel.

Math: out = inputs @ W_eff,  W_eff = sum_r relation_weights[r] * relation_scales[r]
Shapes: inputs [500000, 128] f32, relation_weights [8, 128, 128] f32,
        relation_scales [8, 1] f32  ->  out [500000, 128] f32.

Strategy (data-parallel over 8 NeuronCores, no comm):
  - Pad the node axis to 8 * SHARD rows, one shard per core.
  - Per core: compute W_eff once on-device (DVE scale+add), then stream the
    shard in CHUNK-row DMA chunks. Per 128-node tile: PE transpose (X tile is
    the stationary operand, identity streams) -> X^T in PSUM -> DVE copy to
    SBUF -> PE matmul (lhsT = X^T, rhs = W_eff) -> OUT tile natural layout in
    PSUM -> ACT copy to SBUF -> DMA out. Grouped 4 tiles per PSUM bank so the
    PSUM->SBUF copies are [128, 512].
  - Memory-bound problem: DMA in/out (~64 MB per core) is the roofline.
"""

import numpy as np

N_CORES = 8
D = 128
R = 8
TILE = 128
SUPER = 512               # 4 tiles share one PSUM bank / one copy
CHUNK = 4096              # rows per DMA chunk (2 MiB, 16KB per-partition runs)
SHARD = 62976             # 123 supertiles; 8*62976 = 503808 >= 500000

_CACHE = {}


def _build_nc(shard_rows, chunk_rows):
    import concourse.mybir as mybir
    import concourse.tile as tile
    from concourse import bacc
    from concourse.masks import make_identity

    assert shard_rows % SUPER == 0

    nc = bacc.Bacc()
    x_ext = nc.declare_dram_parameter("x", [shard_rows, D], mybir.dt.float32, isOutput=False)
    rw_ext = nc.declare_dram_parameter("rw", [R, D, D], mybir.dt.float32, isOutput=False)
    rs_ext = nc.declare_dram_parameter("rs", [R, 1], mybir.dt.float32, isOutput=False)
    out_ext = nc.declare_dram_parameter("out", [shard_rows, D], mybir.dt.float32, isOutput=True)

    with tile.TileContext(nc) as tc:
        with (
            tc.tile_pool(name="const", bufs=1) as const_pool,
            tc.tile_pool(name="xf", bufs=3) as xf_pool,
            tc.tile_pool(name="xin", bufs=6) as x_pool,
            tc.tile_pool(name="xt", bufs=6) as xt_pool,
            tc.tile_pool(name="oout", bufs=3) as o_pool,
            tc.tile_pool(name="tpsum", bufs=3, space="PSUM") as tr_pool,
            tc.tile_pool(name="mpsum", bufs=4, space="PSUM") as mm_pool,
            tc.tile_pool(name="wpsum", bufs=1, space="PSUM") as wp_pool,
        ):
            BF16 = mybir.dt.bfloat16
            ident = const_pool.tile([D, D], BF16)
            make_identity(nc, ident[:])
            ident_f = const_pool.tile([D, D], mybir.dt.float32)
            make_identity(nc, ident_f[:])

            # W_eff = sum_r rw[r] * rs[r].
            # Keep DVE (which feeds the steady-state pipeline and is in-order)
            # completely out of the prep: scaled identities on ACT, accumulate
            # via 8 PE matmuls into PSUM, final cast on ACT. Weights arrive on
            # the scalar DMA ring, which is idle early.
            w_all = const_pool.tile([D, R, D], mybir.dt.float32)
            nc.scalar.dma_start(w_all[:], rw_ext[:, :, :].rearrange("r k m -> k r m"))
            s_row = const_pool.tile([1, R], mybir.dt.float32)
            nc.scalar.dma_start(s_row[:], rs_ext[:, :].rearrange("r o -> o r"))
            s_bc = const_pool.tile([D, R], mybir.dt.float32)
            nc.gpsimd.partition_broadcast(s_bc[:], s_row[0:1, :])
            w_ps = wp_pool.tile([D, D], mybir.dt.float32)
            si = [const_pool.tile([D, D], mybir.dt.float32, name=f"si{r}", tag=f"si{r}") for r in range(R)]
            for r in range(R):
                nc.scalar.mul(si[r][:], ident_f[:], s_bc[:, r : r + 1])
            for r in range(R):
                nc.tensor.matmul(w_ps[:], si[r][:], w_all[:, r, :], start=(r == 0), stop=(r == R - 1))
            w_bf = const_pool.tile([D, D], BF16)
            nc.scalar.copy(w_bf[:], w_ps[:])

            # chunk schedule: small chunks at head (fast pipeline ramp) and
            # tail (fast drain), big chunks in the middle for DMA efficiency
            if shard_rows >= 4 * chunk_rows:
                chunks = [SUPER] * 4
                remaining = shard_rows - 4 * SUPER - 3 * SUPER
                mid_n = remaining // chunk_rows
                leftover = remaining - mid_n * chunk_rows
                assert leftover % SUPER == 0 and leftover >= 0
                chunks += [chunk_rows] * mid_n + [SUPER] * (leftover // SUPER) + [SUPER] * 3
            else:
                chunks = []
                r = shard_rows
                while r > 0:
                    c = min(chunk_rows, r)
                    chunks.append(c)
                    r -= c

            c0 = 0
            for rows in chunks:
                assert rows % SUPER == 0
                ntiles = rows // TILE
                nsuper = rows // SUPER

                # layout: partition p holds rows [c0 + p*ntiles, c0 + (p+1)*ntiles)
                # -> per-partition DRAM runs of ntiles*512B (8KB) for the DMA.
                x_f = xf_pool.tile([TILE, ntiles, D], mybir.dt.float32, tag="xf")
                nc.sync.dma_start(
                    x_f[:], x_ext[c0 : c0 + rows, :].rearrange("(p j) d -> p j d", j=ntiles)
                )
                o_t = o_pool.tile([TILE, ntiles, D], mybir.dt.float32, tag="o")

                for s in range(nsuper):
                    x_bf = x_pool.tile([TILE, 4, TILE], BF16, tag="x")
                    nc.vector.tensor_copy(x_bf[:], x_f[:, s * 4 : s * 4 + 4, :])
                    tr_ps = tr_pool.tile([TILE, 4, TILE], BF16, tag="trp")
                    for u in range(4):
                        nc.tensor.transpose(tr_ps[:, u, :], x_bf[:, u, :], ident[:])
                    xt_t = xt_pool.tile([TILE, 4, TILE], BF16, tag="xt")
                    nc.vector.tensor_copy(xt_t[:], tr_ps[:])
                    mm_ps = mm_pool.tile([TILE, 4, TILE], mybir.dt.float32, tag="mmp")
                    for u in range(4):
                        nc.tensor.matmul(mm_ps[:, u, :], xt_t[:, u, :], w_bf[:])
                    nc.scalar.copy(o_t[:, s * 4 : s * 4 + 4, :], mm_ps[:])

                nc.scalar.dma_start(
                    out_ext[c0 : c0 + rows, :].rearrange("(p j) d -> p j d", j=ntiles), o_t[:]
                )
                c0 += rows
            assert c0 == shard_rows

    nc.finalize()
    return nc


def _get_nc(shard_rows=None, chunk_rows=None):
    shard_rows = SHARD if shard_rows is None else shard_rows
    chunk_rows = CHUNK if chunk_rows is None else chunk_rows
    key = (shard_rows, chunk_rows)
    if key not in _CACHE:
        _CACHE[key] = _build_nc(shard_rows, chunk_rows)
    return _CACHE[key]


def _run(inputs, relation_weights, relation_scales, trace=False):
    from concourse.bass_utils import run_bass_kernel_spmd

    x = np.ascontiguousarray(np.asarray(inputs, dtype=np.float32))
    rw = np.ascontiguousarray(np.asarray(relation_weights, dtype=np.float32))
    rs = np.ascontiguousarray(np.asarray(relation_scales, dtype=np.float32))
    n_in = x.shape[0]

    total = SHARD * N_CORES
    assert total >= n_in
    xp = np.zeros((total, D), dtype=np.float32)
    xp[:n_in] = x
    shards = xp.reshape(N_CORES, SHARD, D)

    in_maps = [
        {"x": np.ascontiguousarray(shards[i]), "rw": rw, "rs": rs} for i in range(N_CORES)
    ]
    nc = _get_nc()
    res = run_bass_kernel_spmd(nc, in_maps, core_ids=list(range(N_CORES)), trace=trace)
    out = np.concatenate([res.results[i]["out"] for i in range(N_CORES)], axis=0)[:n_in]
    return out, res


def kernel(inputs, relation_weights, relation_scales):
    out, _ = _run(inputs, relation_weights, relation_scales, trace=False)
    return out


# revision 18
# speedup vs baseline: 1.1051x; 1.1051x over previous
"""
AdaptiveMessagePassingLayer Trainium2 kernel.

Math: out = inputs @ W_eff,  W_eff = sum_r relation_weights[r] * relation_scales[r]
Shapes: inputs [500000, 128] f32, relation_weights [8, 128, 128] f32,
        relation_scales [8, 1] f32  ->  out [500000, 128] f32.

Strategy (data-parallel over 8 NeuronCores, no comm):
  - Pad the node axis to 8 * SHARD rows, one shard per core.
  - Per core: compute W_eff once on-device (DVE scale+add), then stream the
    shard in CHUNK-row DMA chunks. Per 128-node tile: PE transpose (X tile is
    the stationary operand, identity streams) -> X^T in PSUM -> DVE copy to
    SBUF -> PE matmul (lhsT = X^T, rhs = W_eff) -> OUT tile natural layout in
    PSUM -> ACT copy to SBUF -> DMA out. Grouped 4 tiles per PSUM bank so the
    PSUM->SBUF copies are [128, 512].
  - Memory-bound problem: DMA in/out (~64 MB per core) is the roofline.
"""

import numpy as np

N_CORES = 8
D = 128
R = 8
TILE = 128
SUPER = 512               # 4 tiles share one PSUM bank / one copy
CHUNK = 2048              # rows per DMA chunk (1 MiB, 8KB per-partition runs)
SHARD = 62976             # 123 supertiles; 8*62976 = 503808 >= 500000

_CACHE = {}


def _build_nc(shard_rows, chunk_rows):
    import concourse.mybir as mybir
    import concourse.tile as tile
    from concourse import bacc
    from concourse.masks import make_identity

    assert shard_rows % SUPER == 0

    nc = bacc.Bacc()
    x_ext = nc.declare_dram_parameter("x", [shard_rows, D], mybir.dt.float32, isOutput=False)
    rw_ext = nc.declare_dram_parameter("rw", [R, D, D], mybir.dt.float32, isOutput=False)
    rs_ext = nc.declare_dram_parameter("rs", [R, 1], mybir.dt.float32, isOutput=False)
    out_ext = nc.declare_dram_parameter("out", [shard_rows, D], mybir.dt.float32, isOutput=True)

    with tile.TileContext(nc) as tc:
        with (
            tc.tile_pool(name="const", bufs=1) as const_pool,
            tc.tile_pool(name="xf", bufs=4) as xf_pool,
            tc.tile_pool(name="xin", bufs=6) as x_pool,
            tc.tile_pool(name="xt", bufs=6) as xt_pool,
            tc.tile_pool(name="oout", bufs=3) as o_pool,
            tc.tile_pool(name="tpsum", bufs=3, space="PSUM") as tr_pool,
            tc.tile_pool(name="mpsum", bufs=4, space="PSUM") as mm_pool,
            tc.tile_pool(name="wpsum", bufs=1, space="PSUM") as wp_pool,
        ):
            BF16 = mybir.dt.bfloat16
            ident = const_pool.tile([D, D], BF16)
            make_identity(nc, ident[:])
            ident_f = const_pool.tile([D, D], mybir.dt.float32)
            make_identity(nc, ident_f[:])

            # W_eff = sum_r rw[r] * rs[r].
            # Keep DVE (which feeds the steady-state pipeline and is in-order)
            # completely out of the prep: scaled identities on ACT, accumulate
            # via 8 PE matmuls into PSUM, final cast on ACT. Weights arrive on
            # the scalar DMA ring, which is idle early.
            w_all = const_pool.tile([D, R, D], mybir.dt.float32)
            nc.scalar.dma_start(w_all[:], rw_ext[:, :, :].rearrange("r k m -> k r m"))
            s_row = const_pool.tile([1, R], mybir.dt.float32)
            nc.scalar.dma_start(s_row[:], rs_ext[:, :].rearrange("r o -> o r"))
            s_bc = const_pool.tile([D, R], mybir.dt.float32)
            nc.gpsimd.partition_broadcast(s_bc[:], s_row[0:1, :])
            w_ps = wp_pool.tile([D, D], mybir.dt.float32)
            si = [const_pool.tile([D, D], mybir.dt.float32, name=f"si{r}", tag=f"si{r}") for r in range(R)]
            for r in range(R):
                nc.scalar.mul(si[r][:], ident_f[:], s_bc[:, r : r + 1])
            for r in range(R):
                nc.tensor.matmul(w_ps[:], si[r][:], w_all[:, r, :], start=(r == 0), stop=(r == R - 1))
            w_bf = const_pool.tile([D, D], BF16)
            nc.scalar.copy(w_bf[:], w_ps[:])

            # chunk schedule: small chunks at head (fast pipeline ramp) and
            # tail (fast drain), big chunks in the middle for DMA efficiency
            if shard_rows >= 4 * chunk_rows:
                chunks = [SUPER] * 4
                remaining = shard_rows - 4 * SUPER - 3 * SUPER
                mid_n = remaining // chunk_rows
                leftover = remaining - mid_n * chunk_rows
                assert leftover % SUPER == 0 and leftover >= 0
                chunks += [chunk_rows] * mid_n + [SUPER] * (leftover // SUPER) + [SUPER] * 3
            else:
                chunks = []
                r = shard_rows
                while r > 0:
                    c = min(chunk_rows, r)
                    chunks.append(c)
                    r -= c

            c0 = 0
            for rows in chunks:
                assert rows % SUPER == 0
                ntiles = rows // TILE
                nsuper = rows // SUPER

                # layout: partition p holds rows [c0 + p*ntiles, c0 + (p+1)*ntiles)
                # -> per-partition DRAM runs of ntiles*512B (8KB) for the DMA.
                x_f = xf_pool.tile([TILE, ntiles, D], mybir.dt.float32, tag="xf")
                nc.sync.dma_start(
                    x_f[:], x_ext[c0 : c0 + rows, :].rearrange("(p j) d -> p j d", j=ntiles)
                )
                o_t = o_pool.tile([TILE, ntiles, D], mybir.dt.float32, tag="o")

                for s in range(nsuper):
                    x_bf = x_pool.tile([TILE, 4, TILE], BF16, tag="x")
                    nc.vector.tensor_copy(x_bf[:], x_f[:, s * 4 : s * 4 + 4, :])
                    tr_ps = tr_pool.tile([TILE, 4, TILE], BF16, tag="trp")
                    for u in range(4):
                        nc.tensor.transpose(tr_ps[:, u, :], x_bf[:, u, :], ident[:])
                    xt_t = xt_pool.tile([TILE, 4, TILE], BF16, tag="xt")
                    nc.vector.tensor_copy(xt_t[:], tr_ps[:])
                    mm_ps = mm_pool.tile([TILE, 4, TILE], mybir.dt.float32, tag="mmp")
                    for u in range(4):
                        nc.tensor.matmul(mm_ps[:, u, :], xt_t[:, u, :], w_bf[:])
                    nc.scalar.copy(o_t[:, s * 4 : s * 4 + 4, :], mm_ps[:])

                nc.scalar.dma_start(
                    out_ext[c0 : c0 + rows, :].rearrange("(p j) d -> p j d", j=ntiles), o_t[:]
                )
                c0 += rows
            assert c0 == shard_rows

    nc.finalize()
    return nc


def _get_nc(shard_rows=None, chunk_rows=None):
    shard_rows = SHARD if shard_rows is None else shard_rows
    chunk_rows = CHUNK if chunk_rows is None else chunk_rows
    key = (shard_rows, chunk_rows)
    if key not in _CACHE:
        _CACHE[key] = _build_nc(shard_rows, chunk_rows)
    return _CACHE[key]


def _run(inputs, relation_weights, relation_scales, trace=False):
    from concourse.bass_utils import run_bass_kernel_spmd

    x = np.ascontiguousarray(np.asarray(inputs, dtype=np.float32))
    rw = np.ascontiguousarray(np.asarray(relation_weights, dtype=np.float32))
    rs = np.ascontiguousarray(np.asarray(relation_scales, dtype=np.float32))
    n_in = x.shape[0]

    total = SHARD * N_CORES
    assert total >= n_in
    xp = np.zeros((total, D), dtype=np.float32)
    xp[:n_in] = x
    shards = xp.reshape(N_CORES, SHARD, D)

    in_maps = [
        {"x": np.ascontiguousarray(shards[i]), "rw": rw, "rs": rs} for i in range(N_CORES)
    ]
    nc = _get_nc()
    res = run_bass_kernel_spmd(nc, in_maps, core_ids=list(range(N_CORES)), trace=trace)
    out = np.concatenate([res.results[i]["out"] for i in range(N_CORES)], axis=0)[:n_in]
    return out, res


def kernel(inputs, relation_weights, relation_scales):
    out, _ = _run(inputs, relation_weights, relation_scales, trace=False)
    return out
